# revision 1
# baseline (speedup 1.0000x reference)
"""PositionalSparseLinear v4: pair-pooled dedup gather via indirect DMA +
compressed scatter-matrix PE accumulation.

Same algorithm as v3 but the pool gather uses one indirect_dma_start per
128-row chunk (idx [128,1] int32), since custom-ucode dma_gather does not
compile in this environment. Each gather group (pair; pair 0 split in two
halves) owns a dedicated semaphore and consumers wait only for a group's
full count, so unordered DMA completions cannot satisfy a wait early.
"""

import sys

sys.path.insert(0, "/opt/trn_rl_repo")

import numpy as np

from contextlib import ExitStack

import concourse.bass as bass
import concourse.mybir as mybir
from concourse.bass_utils import run_bass_kernel_spmd

B = 1024
IN = 8192
O = 8192
K = 32
NCORES = 8
OC = O // NCORES       # 1024
NT = OC // 128         # 8 tiles/core
NP = NT // 2           # 4 pairs/core
NBH = B // 512

F16 = mybir.dt.float16
F32 = mybir.dt.float32
I32 = mybir.dt.int32

_cached = {}


def _build_program(cmax):
    chalf = (cmax + 1) // 2
    nc = bass.Bass()
    xT_in = nc.declare_dram_parameter("xT16", [IN, B], F16, isOutput=False)
    st_in = nc.declare_dram_parameter("stat", [NT, 128, cmax * 128], F16, isOutput=False)
    gi_in = nc.declare_dram_parameter("gidx", [128, NP, cmax], I32, isOutput=False)
    y_out = nc.declare_dram_parameter("y", [NT, 128, B], F32, isOutput=True)

    with (
        nc.sbuf_tensor("pool_sb", [128, 2, cmax, B], F16) as pool_sb,
        nc.sbuf_tensor("st_sb", [128, 2, cmax * 128], F16) as st_sb,
        nc.sbuf_tensor("gi_sb", [128, NP, cmax], I32) as gi_sb,
        nc.sbuf_tensor("out_sb", [128, 2, B], F32) as out_sb,
        ExitStack() as _stack,
        nc.Block() as block,
        nc.semaphore("i_sem") as i_sem,
        nc.semaphore("gh0a") as gh0a,        # pair 0 first half
        nc.semaphore("gh0b") as gh0b,        # pair 0 second half
        nc.semaphore("g1") as g1,
        nc.semaphore("g2") as g2,
        nc.semaphore("g3") as g3,
        nc.semaphore("st_sem0") as st_sem0,
        nc.semaphore("st_sem1") as st_sem1,
        nc.semaphore("pe_sem") as pe_sem,
        nc.semaphore("v_sem") as v_sem,
        nc.semaphore("yd_sem0") as yd_sem0,
        nc.semaphore("yd_sem1") as yd_sem1,
    ):
        _ps = [
            _stack.enter_context(nc.psum_tensor(f"ps{i}", [128, 512], F32))
            for i in range(8)
        ]
        psum = [(_ps[0], _ps[1]), (_ps[2], _ps[3]), (_ps[4], _ps[5]), (_ps[6], _ps[7])]
        st_sems = [st_sem0, st_sem1]
        yd_sems = [yd_sem0, yd_sem1]
        pair_sems = [None, g1, g2, g3]

        @block.sync
        def _(sync: bass.BassEngine):
            sync.dma_start(out=gi_sb[:], in_=gi_in[:]).then_inc(i_sem, 16)
            for T in range(NT):
                if T >= 2:
                    sync.wait_ge(pe_sem, T - 1)
                sync.dma_start(out=st_sb[:, T % 2], in_=st_in[T]).then_inc(
                    st_sems[T % 2], 16
                )

        @block.gpsimd
        def _(gpsimd: bass.BassGpSimd):
            gpsimd.wait_ge(i_sem, 16)
            for p in range(NP):
                if p >= 2:
                    gpsimd.wait_ge(pe_sem, 2 * p - 2)
                for cn in range(cmax):
                    if p == 0:
                        sem = gh0a if cn < chalf else gh0b
                    else:
                        sem = pair_sems[p]
                    gpsimd.indirect_dma_start(
                        out=pool_sb[:, p % 2, cn],
                        out_offset=None,
                        in_=xT_in[:],
                        in_offset=bass.IndirectOffsetOnAxis(
                            ap=gi_sb[:, p, cn:cn + 1], axis=0
                        ),
                    ).then_inc(sem, 16)

        @block.tensor
        def _(pe: bass.BassEngine):
            for T in range(NT):
                p = T // 2
                if T >= 4:
                    pe.wait_ge(v_sem, T - 3)
                pe.wait_ge(st_sems[T % 2], 16 * (T // 2 + 1))
                if p == 0:
                    pe.wait_ge(gh0a, 16 * chalf)
                else:
                    pe.wait_ge(pair_sems[p], 16 * cmax)
                for bh in range(NBH):
                    for cn in range(cmax):
                        if p == 0 and cn == chalf and bh == 0 and T == 0:
                            pe.wait_ge(gh0b, 16 * (cmax - chalf))
                        mm = pe.matmul(
                            out=psum[T % 4][bh][:],
                            lhsT=st_sb[:, T % 2, cn * 128:(cn + 1) * 128],
                            rhs=pool_sb[:, p % 2, cn, bh * 512:(bh + 1) * 512],
                            start=(cn == 0),
                            stop=(cn == cmax - 1),
                        )
                        if bh == NBH - 1 and cn == cmax - 1:
                            mm.then_inc(pe_sem, 1)

        @block.vector
        def _(vector: bass.BassEngine):
            for T in range(NT):
                vector.wait_ge(pe_sem, T + 1)
                if T >= 2:
                    vector.wait_ge(yd_sems[T % 2], 16 * (T // 2))
                vector.tensor_copy(out=out_sb[:, T % 2, 0:512], in_=psum[T % 4][0][:])
                vector.tensor_copy(
                    out=out_sb[:, T % 2, 512:1024], in_=psum[T % 4][1][:]
                ).then_inc(v_sem, 1)

        @block.scalar
        def _(scalar: bass.BassEngine):
            for T in range(NT):
                scalar.wait_ge(v_sem, T + 1)
                scalar.dma_start(out=y_out[T], in_=out_sb[:, T % 2]).then_inc(
                    yd_sems[T % 2], 16
                )
            scalar.wait_ge(yd_sems[0], 16 * (NT // 2))
            scalar.wait_ge(yd_sems[1], 16 * (NT // 2))

    return nc


def _prep_inputs(x, connections, weights):
    xT16 = np.ascontiguousarray(x.T.astype(np.float16))        # [IN, B]

    conn = connections.reshape(NCORES, NP, 256, K)
    wts = weights.reshape(NCORES, NP, 256, K).astype(np.float32)

    uniqs = [[np.unique(conn[c, p]) for p in range(NP)] for c in range(NCORES)]
    cmax = max((len(u) + 127) // 128 for per_core in uniqs for u in per_core)

    gidx = np.zeros((NCORES, 128, NP, cmax), dtype=np.int32)
    stat = np.zeros((NCORES, NT, 128, cmax * 128), dtype=np.float16)
    for c in range(NCORES):
        for p in range(NP):
            u = uniqs[c][p]
            n_u = len(u)
            pool = np.zeros(cmax * 128, dtype=np.int64)
            pool[:n_u] = u
            # slot (cn, s): idx[s, p, cn] = pool[cn*128 + s]
            gidx[c, :, p, :] = pool.reshape(cmax, 128).T
            slots = np.searchsorted(u, conn[c, p])                 # [256, K]
            st = np.zeros((2, cmax * 128, 128), dtype=np.float32)  # [tt, slot, m]
            tt = np.repeat(np.arange(256) // 128, K).reshape(256, K)
            m = np.repeat(np.arange(256) % 128, K).reshape(256, K)
            np.add.at(st, (tt, slots, m), wts[c, p])
            for ti in range(2):
                stat[c, 2 * p + ti] = (
                    st[ti].astype(np.float16)
                    .reshape(cmax, 128, 128)       # [cn, s, m]
                    .transpose(1, 0, 2)            # [s, cn, m]
                    .reshape(128, cmax * 128)
                )
    return xT16, stat, gidx, cmax


def kernel(x, connections, weights):
    x = np.asarray(x)
    connections = np.asarray(connections)
    weights = np.asarray(weights)
    xT16, stat, gidx, cmax = _prep_inputs(x, connections, weights)
    if cmax not in _cached:
        _cached[cmax] = _build_program(cmax)
    nc = _cached[cmax]
    in_maps = [
        {"xT16": xT16, "stat": stat[c], "gidx": gidx[c]} for c in range(NCORES)
    ]
    res = run_bass_kernel_spmd(nc, in_maps, core_ids=list(range(NCORES)))
    out = np.empty((B, O), dtype=np.float32)
    for c in range(NCORES):
        y = res.results[c]["y"]
        out[:, c * OC:(c + 1) * OC] = y.reshape(OC, B).T
    return out



# revision 3
# speedup vs baseline: 1.2991x; 1.2991x over previous
"""PositionalSparseLinear v5: hybrid slab + indirect gather, quad-level dedup
pools with class-sorted layout, per-tile trimmed scatter-matrix matmuls.

Per core (1024 outs = 8 tiles of 128), outputs split into two quads of 4
tiles. Each quad's needed input rows (dedup'd) form a pool, ordered by the
4-bit "which tiles use this row" class mask in a fixed prefix-staggered
sequence so each tile's rows occupy a small number of 128-row chunks
(tile A: 1 run, B: 1, C: 2, D: 4 — total runs 8, the DP optimum).

Quad 0's pool is made a CONTIGUOUS slab of a per-core row-permuted copy of
xT (permutation depends only on `connections`), so it loads with plain
HWDGE dma_starts — no Pool-engine descriptor generation. Quad 1 gathers
per-chunk via indirect DMA (128 rows/op, the only shape the HW supports).
Both land in a 72-slot ring pool; quad 1 reuses quad 0's slots gated on
per-tile PE progress. Per-tile stat matrices only cover the chunks in that
tile's class window, cutting PE work ~1.5x vs pair-level pooling.

SPMD constraint: class-region capacities are fixed to the max over cores
(compile-time), so all 8 cores share one program; per-core data (stat
values, gather indices, permuted x) fills the regions.
"""

import sys

sys.path.insert(0, "/opt/trn_rl_repo")

import numpy as np

from contextlib import ExitStack

import concourse.bass as bass
import concourse.mybir as mybir
from concourse.bass_utils import run_bass_kernel_spmd

B = 1024
IN = 8192
O = 8192
K = 32
NCORES = 8
OC = O // NCORES       # 1024
NT = OC // 128         # 8 tiles/core
RING = 72              # pool ring slots (x2KB/partition = 144KB)
GRP = 8                # chunks per gather group / slab sub-op

F16 = mybir.dt.float16
F32 = mybir.dt.float32
I32 = mybir.dt.int32

# Class order for 4-bit tile-usage masks (bit t = tile t uses the row).
# Prefix-staggered: tile0's 8 classes first (1 run), tile1 contiguous
# (1 run), tile2 2 runs, tile3 4 runs; staggered finish positions.
CLASS_ORDER = [1, 9, 13, 5, 7, 15, 11, 3, 2, 10, 14, 6, 4, 12, 8]

_cached = {}
_static_prep = {}


def _tile_windows(caps):
    """Per-tile (row_lo, row_hi) run list over class regions with fixed
    capacities `caps` (len 15). Returns [tile][(lo, hi)...] in rows."""
    offs = np.concatenate([[0], np.cumsum(caps)])
    wins = []
    for t in range(4):
        runs = []
        for i, m in enumerate(CLASS_ORDER):
            if (m >> t) & 1:
                lo, hi = offs[i], offs[i + 1]
                if runs and runs[-1][1] == lo:
                    runs[-1] = (runs[-1][0], hi)
                else:
                    runs.append((lo, hi))
        wins.append(runs)
    return wins


def _chunks_of(runs):
    """Distinct 128-row chunk indices covering the row runs, ascending."""
    cs = set()
    for lo, hi in runs:
        cs.update(range(lo // 128, (hi + 127) // 128))
    return sorted(cs)


def _prep_static(connections):
    """Everything that depends only on `connections` (static weights side
    minus the actual weight values): permutations, gather indices, pool
    slot maps, and the shared program structure."""
    conn = connections.reshape(NCORES, NT, 128, K)

    # per (core, quad): usage class of each input row
    used = np.zeros((NCORES, 2, IN), dtype=np.uint8)
    for c in range(NCORES):
        for T in range(NT):
            q, t = divmod(T, 4)
            rows = np.unique(conn[c, T])
            used[c, q, rows] |= np.uint8(1 << t)

    # fixed class capacities per quad index (max over cores)
    caps = np.zeros((2, 15), dtype=np.int64)
    for q in range(2):
        for i, m in enumerate(CLASS_ORDER):
            caps[q, i] = max(
                int((used[c, q] == m).sum()) for c in range(NCORES)
            )
    L = [int(caps[q].sum()) for q in range(2)]
    n = [(L[q] + 127) // 128 for q in range(2)]
    n0, n1 = n
    assert n0 <= RING and n1 <= RING and max(n0, n1) <= RING, (n0, n1)

    wins = [_tile_windows(caps[q]) for q in range(2)]
    # global tile chunk lists and ring slots
    tile_chunks = []            # [T] -> list of quad-chunk indices
    tile_ring = []              # [T] -> list of ring slots
    for T in range(NT):
        q, t = divmod(T, 4)
        cs = _chunks_of(wins[q][t])
        tile_chunks.append(cs)
        if q == 0:
            tile_ring.append(cs)
        else:
            tile_ring.append([(n0 + j) % RING for j in cs])
    k_T = [len(cs) for cs in tile_chunks]
    TOT = sum(k_T)
    offs_T = np.concatenate([[0], np.cumsum(k_T)])

    # ring-reuse thresholds: q0 chunk s's last consumer tile (global idx)+1
    thr = np.zeros(n0, dtype=np.int64)
    for t in range(4):
        for j in tile_chunks[t]:
            thr[j] = max(thr[j], t + 1)

    # per-core pools, permutation, gather indices
    pools = np.zeros((NCORES, 2, max(n0, n1) * 128), dtype=np.int64)
    perms = np.zeros((NCORES, IN), dtype=np.int64)
    gidx = np.zeros((NCORES, 128, n1), dtype=np.int32)
    slot = np.full((NCORES, 2, IN), -1, dtype=np.int64)
    for c in range(NCORES):
        for q in range(2):
            Lq = n[q] * 128
            pool = np.zeros(Lq, dtype=np.int64)
            fill = np.zeros(Lq, dtype=bool)
            off = 0
            for i, m in enumerate(CLASS_ORDER):
                rows = np.flatnonzero(used[c, q] == m)
                pool[off:off + len(rows)] = rows
                fill[off:off + len(rows)] = True
                off += int(caps[q, i])
            unused = np.flatnonzero(used[c, q] == 0)
            npad = int((~fill).sum())
            assert npad <= len(unused), (c, q, npad, len(unused))
            pool[~fill] = unused[:npad]
            pools[c, q, :Lq] = pool
            real = np.flatnonzero(fill)
            slot[c, q, pool[real]] = real
        # permutation: quad0 pool first, then the rest
        p0 = pools[c, 0, :n0 * 128]
        inp0 = np.zeros(IN, dtype=bool)
        inp0[p0] = True
        perms[c] = np.concatenate([p0, np.flatnonzero(~inp0)])
        pos = np.empty(IN, dtype=np.int64)
        pos[perms[c]] = np.arange(IN)
        gidx[c] = pos[pools[c, 1, :n1 * 128]].reshape(n1, 128).T

    # gather groups for q1: [(j_lo, j_hi)], sem group per GRP ops
    q1_groups = [(a, min(a + GRP, n1)) for a in range(0, n1, GRP)]
    # slab sub-ops for q0
    q0_subs = [(a, min(a + GRP, n0)) for a in range(0, n0, GRP)]

    struct = dict(
        n0=n0, n1=n1, k_T=k_T, TOT=TOT,
        offs_T=[int(v) for v in offs_T],
        tile_ring=tile_ring, tile_chunks=tile_chunks,
        thr=[int(v) for v in thr],
        q0_subs=q0_subs, q1_groups=q1_groups,
        NST=max(k_T),
    )
    return dict(
        conn=conn, used=used, caps=caps, struct=struct, pools=pools,
        perms=perms, gidx=gidx, slot=slot,
    )


def _prep_stat(sp, weights):
    """Per-core packed stat matrices [128, TOT*128] f16."""
    st = sp["struct"]
    conn, slot = sp["conn"], sp["slot"]
    wts = weights.reshape(NCORES, NT, 128, K).astype(np.float32)
    n = [st["n0"], st["n1"]]
    TOT = st["TOT"]
    stat = np.zeros((NCORES, 128, TOT * 128), dtype=np.float16)
    m_idx = np.repeat(np.arange(128), K).reshape(128, K)
    for c in range(NCORES):
        for T in range(NT):
            q = T // 4
            s = slot[c, q][conn[c, T]]          # [128, K]
            assert (s >= 0).all()
            acc = np.zeros((n[q] * 128, 128), dtype=np.float32)
            np.add.at(acc, (s, m_idx), wts[c, T])
            blk = acc.reshape(n[q], 128, 128)[st["tile_chunks"][T]]
            blk = blk.transpose(1, 0, 2).reshape(128, -1)   # [s, k*128]
            o = st["offs_T"][T] * 128
            stat[c, :, o:o + blk.shape[1]] = blk.astype(np.float16)
    return stat


def _build_program(st):
    n0, n1 = st["n0"], st["n1"]
    NST = st["NST"]
    nc = bass.Bass()
    xTp_in = nc.declare_dram_parameter("xTp", [IN, B], F16, isOutput=False)
    st_in = nc.declare_dram_parameter("stat", [128, st["TOT"] * 128], F16,
                                      isOutput=False)
    gi_in = nc.declare_dram_parameter("gidx", [128, n1], I32, isOutput=False)
    y_out = nc.declare_dram_parameter("y", [NT, 128, B], F16, isOutput=True)

    nga = len(st["q0_subs"])
    ngb = len(st["q1_groups"])

    with (
        nc.sbuf_tensor("pool_sb", [128, RING, B], F16) as pool_sb,
        nc.sbuf_tensor("st_sb", [128, 2, NST * 128], F16) as st_sb,
        nc.sbuf_tensor("gi_sb", [128, n1], I32) as gi_sb,
        nc.sbuf_tensor("out_sb", [128, 2, B], F16) as out_sb,
        ExitStack() as _stack,
        nc.Block() as block,
        nc.semaphore("i_sem") as i_sem,
        nc.semaphore("pe_sem") as pe_sem,
        nc.semaphore("v_sem") as v_sem,
        nc.semaphore("st_sem0") as st_sem0,
        nc.semaphore("st_sem1") as st_sem1,
        nc.semaphore("yd_sem0") as yd_sem0,
        nc.semaphore("yd_sem1") as yd_sem1,
    ):
        ga = [_stack.enter_context(nc.semaphore(f"ga{i}")) for i in range(nga)]
        gb = [_stack.enter_context(nc.semaphore(f"gb{i}")) for i in range(ngb)]
        _ps = [
            _stack.enter_context(nc.psum_tensor(f"ps{i}", [128, 512], F32))
            for i in range(8)
        ]
        psum = [(_ps[0], _ps[1]), (_ps[2], _ps[3]), (_ps[4], _ps[5]),
                (_ps[6], _ps[7])]
        st_sems = [st_sem0, st_sem1]
        yd_sems = [yd_sem0, yd_sem1]

        # q0 slab: plain HWDGE loads from the permuted x copy
        @block.sync
        def _(sync: bass.BassEngine):
            for k, (a, b) in enumerate(st["q0_subs"]):
                w = b - a
                src = xTp_in[a * 128:b * 128].rearrange(
                    "(j s) b -> s j b", s=128)
                sync.dma_start(
                    out=pool_sb[:, a:b, :], in_=src
                ).then_inc(ga[k], 16)

        # q1 gather: per-chunk indirect DMA into ring slots
        @block.gpsimd
        def _(gpsimd: bass.BassGpSimd):
            gpsimd.dma_start(out=gi_sb[:], in_=gi_in[:]).then_inc(i_sem, 16)
            gpsimd.wait_ge(i_sem, 16)
            last_thr = 0
            for j in range(n1):
                slot = (n0 + j) % RING
                thr = st["thr"][slot] if slot < n0 else 0
                if thr > last_thr:
                    gpsimd.wait_ge(pe_sem, thr)
                    last_thr = thr
                gpsimd.indirect_dma_start(
                    out=pool_sb[:, slot, :],
                    out_offset=None,
                    in_=xTp_in[:],
                    in_offset=bass.IndirectOffsetOnAxis(
                        ap=gi_sb[:, j:j + 1], axis=0
                    ),
                ).then_inc(gb[j // GRP], 16)

        @block.tensor
        def _(pe: bass.BassEngine):
            for T in range(NT):
                q = T // 4
                kT = st["k_T"][T]
                chunks = st["tile_chunks"][T]
                ring = st["tile_ring"][T]
                if T >= 4:
                    pe.wait_ge(v_sem, T - 3)
                pe.wait_ge(st_sems[T % 2], 16 * (T // 2 + 1))
                waited = -1
                for bh in range(2):
                    for k in range(kT):
                        if bh == 0:
                            g = chunks[k] // GRP
                            if q == 0 and g > waited:
                                pe.wait_ge(ga[g], 16)
                                waited = g
                            elif q == 1 and g > waited:
                                a, b_ = st["q1_groups"][g]
                                pe.wait_ge(gb[g], 16 * (b_ - a))
                                waited = g
                        mm = pe.matmul(
                            out=psum[T % 4][bh][:],
                            lhsT=st_sb[:, T % 2, k * 128:(k + 1) * 128],
                            rhs=pool_sb[:, ring[k], bh * 512:(bh + 1) * 512],
                            start=(k == 0),
                            stop=(k == kT - 1),
                        )
                        if bh == 1 and k == kT - 1:
                            mm.then_inc(pe_sem, 1)

        @block.vector
        def _(vector: bass.BassEngine):
            for T in range(NT):
                vector.wait_ge(pe_sem, T + 1)
                if T >= 2:
                    vector.wait_ge(yd_sems[T % 2], 16 * (T // 2))
                vector.tensor_copy(out=out_sb[:, T % 2, 0:512],
                                   in_=psum[T % 4][0][:])
                vector.tensor_copy(
                    out=out_sb[:, T % 2, 512:1024], in_=psum[T % 4][1][:]
                ).then_inc(v_sem, 1)

        # stat loads + output stores share the Activation HWDGE
        @block.scalar
        def _(scalar: bass.BassEngine):
            kT = st["k_T"]
            offs = st["offs_T"]
            for T in range(NT):
                if T >= 2:
                    scalar.wait_ge(pe_sem, T - 1)
                scalar.dma_start(
                    out=st_sb[:, T % 2, 0:kT[T] * 128],
                    in_=st_in[:, offs[T] * 128:(offs[T] + kT[T]) * 128],
                ).then_inc(st_sems[T % 2], 16)
                if T >= 2:
                    Ty = T - 2
                    scalar.wait_ge(v_sem, Ty + 1)
                    scalar.dma_start(
                        out=y_out[Ty], in_=out_sb[:, Ty % 2]
                    ).then_inc(yd_sems[Ty % 2], 16)
            for Ty in (NT - 2, NT - 1):
                scalar.wait_ge(v_sem, Ty + 1)
                scalar.dma_start(
                    out=y_out[Ty], in_=out_sb[:, Ty % 2]
                ).then_inc(yd_sems[Ty % 2], 16)
            scalar.wait_ge(yd_sems[0], 16 * (NT // 2))
            scalar.wait_ge(yd_sems[1], 16 * (NT // 2))

    return nc


def _struct_key(st):
    return (
        st["n0"], st["n1"], tuple(st["k_T"]),
        tuple(tuple(r) for r in st["tile_ring"]),
        tuple(st["thr"]),
    )


def kernel(x, connections, weights):
    x = np.asarray(x)
    connections = np.asarray(connections)
    weights = np.asarray(weights)

    ckey = (connections.tobytes(), weights.tobytes())
    sp = _static_prep.get(ckey)
    if sp is None:
        sp = _prep_static(connections)
        sp["stat"] = _prep_stat(sp, weights)
        _static_prep.clear()
        _static_prep[ckey] = sp
    st = sp["struct"]

    xT16 = np.ascontiguousarray(x.T.astype(np.float16))        # [IN, B]
    key = _struct_key(st)
    if key not in _cached:
        _cached[key] = _build_program(st)
    nc = _cached[key]

    in_maps = [
        {
            "xTp": np.ascontiguousarray(xT16[sp["perms"][c]]),
            "stat": sp["stat"][c],
            "gidx": sp["gidx"][c],
        }
        for c in range(NCORES)
    ]
    res = run_bass_kernel_spmd(nc, in_maps, core_ids=list(range(NCORES)))
    out = np.empty((B, O), dtype=np.float32)
    for c in range(NCORES):
        y = res.results[c]["y"]                    # [NT, 128, B] f16
        out[:, c * OC:(c + 1) * OC] = (
            y.astype(np.float32).reshape(OC, B).T
        )
    return out


# revision 11
# speedup vs baseline: 1.6721x; 1.2871x over previous
"""PositionalSparseLinear v5: hybrid slab + indirect gather, quad-level dedup
pools with class-sorted layout, per-tile trimmed scatter-matrix matmuls.

Per core (1024 outs = 8 tiles of 128), outputs split into two quads of 4
tiles. Each quad's needed input rows (dedup'd) form a pool, ordered by the
4-bit "which tiles use this row" class mask in a fixed prefix-staggered
sequence so each tile's rows occupy a small number of 128-row chunks
(tile A: 1 run, B: 1, C: 2, D: 4 — total runs 8, the DP optimum).

Quad 0's pool is made a CONTIGUOUS slab of a per-core row-permuted copy of
xT (permutation depends only on `connections`), so it loads with plain
HWDGE dma_starts — no Pool-engine descriptor generation. Quad 1 gathers
per-chunk via indirect DMA (128 rows/op, the only shape the HW supports).
Both land in a 72-slot ring pool; quad 1 reuses quad 0's slots gated on
per-tile PE progress. Per-tile stat matrices only cover the chunks in that
tile's class window, cutting PE work ~1.5x vs pair-level pooling.

SPMD constraint: class-region capacities are fixed to the max over cores
(compile-time), so all 8 cores share one program; per-core data (stat
values, gather indices, permuted x) fills the regions.
"""

import sys

sys.path.insert(0, "/opt/trn_rl_repo")

import numpy as np

from contextlib import ExitStack

import concourse.bass as bass
import concourse.mybir as mybir
from concourse.bass_utils import run_bass_kernel_spmd

B = 1024
IN = 8192
O = 8192
K = 32
NCORES = 8
OC = O // NCORES       # 1024
NT = OC // 128         # 8 tiles/core
RING = 72              # pool ring slots (x2KB/partition = 144KB)
GRP = 8                # chunks per gather group / slab sub-op

F16 = mybir.dt.float16
F32 = mybir.dt.float32
I32 = mybir.dt.int32

# Class order for 4-bit tile-usage masks (bit t = tile t uses the row).
# Prefix-staggered: tile0's 8 classes first (1 run), tile1 contiguous
# (1 run), tile2 2 runs, tile3 4 runs; staggered finish positions.
CLASS_ORDER = [1, 9, 13, 5, 7, 15, 11, 3, 2, 10, 14, 6, 4, 12, 8]

_cached = {}
_static_prep = {}


def _tile_windows(caps):
    """Per-tile (row_lo, row_hi) run list over class regions with fixed
    capacities `caps` (len 15). Returns [tile][(lo, hi)...] in rows."""
    offs = np.concatenate([[0], np.cumsum(caps)])
    wins = []
    for t in range(4):
        runs = []
        for i, m in enumerate(CLASS_ORDER):
            if (m >> t) & 1:
                lo, hi = offs[i], offs[i + 1]
                if runs and runs[-1][1] == lo:
                    runs[-1] = (runs[-1][0], hi)
                else:
                    runs.append((lo, hi))
        wins.append(runs)
    return wins


def _chunks_of(runs):
    """Distinct 128-row chunk indices covering the row runs, ascending."""
    cs = set()
    for lo, hi in runs:
        cs.update(range(lo // 128, (hi + 127) // 128))
    return sorted(cs)


def _prep_static(connections):
    """Everything that depends only on `connections` (static weights side
    minus the actual weight values): permutations, gather indices, pool
    slot maps, and the shared program structure."""
    conn = connections.reshape(NCORES, NT, 128, K)

    # per (core, quad): usage class of each input row
    used = np.zeros((NCORES, 2, IN), dtype=np.uint8)
    for c in range(NCORES):
        for T in range(NT):
            q, t = divmod(T, 4)
            rows = np.unique(conn[c, T])
            used[c, q, rows] |= np.uint8(1 << t)

    # fixed class capacities per quad index (max over cores)
    caps = np.zeros((2, 15), dtype=np.int64)
    for q in range(2):
        for i, m in enumerate(CLASS_ORDER):
            caps[q, i] = max(
                int((used[c, q] == m).sum()) for c in range(NCORES)
            )
    L = [int(caps[q].sum()) for q in range(2)]
    n = [(L[q] + 127) // 128 for q in range(2)]
    n0, n1 = n
    assert n0 <= RING and n1 <= RING and max(n0, n1) <= RING, (n0, n1)

    wins = [_tile_windows(caps[q]) for q in range(2)]
    # global tile chunk lists and ring slots
    tile_chunks = []            # [T] -> list of quad-chunk indices
    tile_ring = []              # [T] -> list of ring slots
    for T in range(NT):
        q, t = divmod(T, 4)
        cs = _chunks_of(wins[q][t])
        tile_chunks.append(cs)
        if q == 0:
            tile_ring.append(cs)
        else:
            tile_ring.append([(n0 + j) % RING for j in cs])
    k_T = [len(cs) for cs in tile_chunks]
    TOT = sum(k_T)
    offs_T = np.concatenate([[0], np.cumsum(k_T)])

    # ring-reuse thresholds: q0 chunk s's last consumer tile (global idx)+1
    thr = np.zeros(n0, dtype=np.int64)
    for t in range(4):
        for j in tile_chunks[t]:
            thr[j] = max(thr[j], t + 1)

    # q0 slab sub-ops: small first sub-op for an early PE start
    q0_subs = []
    a = 0
    for w in [2, 4] + [GRP] * 100:
        if a >= n0:
            break
        q0_subs.append((a, min(a + w, n0)))
        a += w
    sub_of_chunk0 = np.zeros(n0, dtype=np.int64)
    for k, (a, b) in enumerate(q0_subs):
        sub_of_chunk0[a:b] = k

    # ring slot assignment for q1 chunks: fresh slots first, then q0 slots
    # sorted by (last-consumer thr, chunk) so gather gating is monotone in
    # consumption order. Each q0-region slot is also gated on the slab
    # sub-op that writes it (write-write race otherwise).
    fresh = list(range(n0, RING))
    q0_slots = sorted(range(n0), key=lambda s: (thr[s], s))
    slot_seq = fresh + q0_slots
    assert n1 <= len(slot_seq)
    q1_slot = slot_seq[:n1]                      # q1 chunk j -> ring slot
    q1_pe_gate = [0 if s >= n0 else int(thr[s]) for s in q1_slot]
    q1_ga_gate = [-1 if s >= n0 else int(sub_of_chunk0[s]) for s in q1_slot]

    # rewrite q1 tiles' ring maps with the assigned slots
    for T in range(4, NT):
        tile_ring[T] = [q1_slot[j] for j in tile_chunks[T]]

    # per-core pools, permutation, gather indices
    pools = np.zeros((NCORES, 2, max(n0, n1) * 128), dtype=np.int64)
    perms = np.zeros((NCORES, IN), dtype=np.int64)
    gidx = np.zeros((NCORES, 128, n1), dtype=np.int32)
    slot = np.full((NCORES, 2, IN), -1, dtype=np.int64)
    for c in range(NCORES):
        for q in range(2):
            Lq = n[q] * 128
            pool = np.zeros(Lq, dtype=np.int64)
            fill = np.zeros(Lq, dtype=bool)
            off = 0
            for i, m in enumerate(CLASS_ORDER):
                rows = np.flatnonzero(used[c, q] == m)
                pool[off:off + len(rows)] = rows
                fill[off:off + len(rows)] = True
                off += int(caps[q, i])
            unused = np.flatnonzero(used[c, q] == 0)
            npad = int((~fill).sum())
            assert npad <= len(unused), (c, q, npad, len(unused))
            pool[~fill] = unused[:npad]
            pools[c, q, :Lq] = pool
            real = np.flatnonzero(fill)
            slot[c, q, pool[real]] = real
        # permutation: quad0 pool first, then the rest
        p0 = pools[c, 0, :n0 * 128]
        inp0 = np.zeros(IN, dtype=bool)
        inp0[p0] = True
        perms[c] = np.concatenate([p0, np.flatnonzero(~inp0)])
        pos = np.empty(IN, dtype=np.int64)
        pos[perms[c]] = np.arange(IN)
        gidx[c] = pos[pools[c, 1, :n1 * 128]].reshape(n1, 128).T

    # gather groups for q1: [(j_lo, j_hi)], sem group per GRP ops
    q1_groups = [(a, min(a + GRP, n1)) for a in range(0, n1, GRP)]

    struct = dict(
        n0=n0, n1=n1, k_T=k_T, TOT=TOT,
        offs_T=[int(v) for v in offs_T],
        tile_ring=tile_ring, tile_chunks=tile_chunks,
        q1_pe_gate=q1_pe_gate, q1_ga_gate=q1_ga_gate,
        q1_slot=[int(s) for s in q1_slot],
        sub_of_chunk0=[int(v) for v in sub_of_chunk0],
        q0_subs=q0_subs, q1_groups=q1_groups,
        NST=max(k_T),
    )
    return dict(
        conn=conn, used=used, caps=caps, struct=struct, pools=pools,
        perms=perms, gidx=gidx, slot=slot,
    )


def _prep_stat(sp, weights):
    """Per-core packed stat matrices [128, TOT*128] f16."""
    st = sp["struct"]
    conn, slot = sp["conn"], sp["slot"]
    wts = weights.reshape(NCORES, NT, 128, K).astype(np.float32)
    n = [st["n0"], st["n1"]]
    TOT = st["TOT"]
    stat = np.zeros((NCORES, 128, TOT * 128), dtype=np.float16)
    m_idx = np.repeat(np.arange(128), K).reshape(128, K)
    for c in range(NCORES):
        for T in range(NT):
            q = T // 4
            s = slot[c, q][conn[c, T]]          # [128, K]
            assert (s >= 0).all()
            acc = np.zeros((n[q] * 128, 128), dtype=np.float32)
            np.add.at(acc, (s, m_idx), wts[c, T])
            blk = acc.reshape(n[q], 128, 128)[st["tile_chunks"][T]]
            blk = blk.transpose(1, 0, 2).reshape(128, -1)   # [s, k*128]
            o = st["offs_T"][T] * 128
            stat[c, :, o:o + blk.shape[1]] = blk.astype(np.float16)
    return stat


def _build_program(st):
    n0, n1 = st["n0"], st["n1"]
    NST = st["NST"]
    nc = bass.Bass()
    xTp_in = nc.declare_dram_parameter("xTp", [IN, B], F16, isOutput=False)
    st_in = nc.declare_dram_parameter("stat", [128, st["TOT"] * 128], F16,
                                      isOutput=False)
    gi_in = nc.declare_dram_parameter("gidx", [128, n1], I32, isOutput=False)
    y_out = nc.declare_dram_parameter("y", [NT, 128, B], F16, isOutput=True)

    nga = len(st["q0_subs"])
    ngb = len(st["q1_groups"])

    with (
        nc.sbuf_tensor("pool_sb", [128, RING, B], F16) as pool_sb,
        nc.sbuf_tensor("st_sb", [128, 2, NST * 128], F16) as st_sb,
        nc.sbuf_tensor("gi_sb", [128, n1], I32) as gi_sb,
        nc.sbuf_tensor("out_sb", [128, 2, B], F16) as out_sb,
        ExitStack() as _stack,
        nc.Block() as block,
        nc.semaphore("i_sem") as i_sem,
        nc.semaphore("pe_sem") as pe_sem,
        nc.semaphore("v_sem") as v_sem,
        nc.semaphore("st_sem0") as st_sem0,
        nc.semaphore("st_sem1") as st_sem1,
        nc.semaphore("yd_sem0") as yd_sem0,
        nc.semaphore("yd_sem1") as yd_sem1,
    ):
        ga = [_stack.enter_context(nc.semaphore(f"ga{i}")) for i in range(nga)]
        gb = [_stack.enter_context(nc.semaphore(f"gb{i}")) for i in range(ngb)]
        _ps = [
            _stack.enter_context(nc.psum_tensor(f"ps{i}", [128, 512], F32))
            for i in range(8)
        ]
        psum = [(_ps[0], _ps[1]), (_ps[2], _ps[3]), (_ps[4], _ps[5]),
                (_ps[6], _ps[7])]
        st_sems = [st_sem0, st_sem1]
        yd_sems = [yd_sem0, yd_sem1]

        # q0 slab: plain HWDGE loads from the permuted x copy
        @block.sync
        def _(sync: bass.BassEngine):
            for k, (a, b) in enumerate(st["q0_subs"]):
                w = b - a
                src = xTp_in[a * 128:b * 128].rearrange(
                    "(j s) b -> s j b", s=128)
                sync.dma_start(
                    out=pool_sb[:, a:b, :], in_=src
                ).then_inc(ga[k], 16)

        # q1 gather: per-chunk indirect DMA into assigned ring slots
        @block.gpsimd
        def _(gpsimd: bass.BassGpSimd):
            gpsimd.dma_start(out=gi_sb[:], in_=gi_in[:]).then_inc(i_sem, 16)
            gpsimd.wait_ge(i_sem, 16)
            last_thr = 0
            waited_ga = set()
            for j in range(n1):
                s = st["q1_slot"][j]
                thr = st["q1_pe_gate"][j]
                gareq = st["q1_ga_gate"][j]
                if gareq >= 0 and gareq not in waited_ga:
                    gpsimd.wait_ge(ga[gareq], 16)
                    waited_ga.add(gareq)
                if thr > last_thr:
                    gpsimd.wait_ge(pe_sem, thr)
                    last_thr = thr
                gpsimd.indirect_dma_start(
                    out=pool_sb[:, s, :],
                    out_offset=None,
                    in_=xTp_in[:],
                    in_offset=bass.IndirectOffsetOnAxis(
                        ap=gi_sb[:, j:j + 1], axis=0
                    ),
                ).then_inc(gb[j // GRP], 16)

        @block.tensor
        def _(pe: bass.BassEngine):
            for T in range(NT):
                q = T // 4
                kT = st["k_T"][T]
                chunks = st["tile_chunks"][T]
                ring = st["tile_ring"][T]
                if T >= 4:
                    pe.wait_ge(v_sem, T - 3)
                pe.wait_ge(st_sems[T % 2], 16 * (T // 2 + 1))
                waited = -1
                for bh in range(2):
                    for k in range(kT):
                        if bh == 0:
                            if q == 0:
                                g = st["sub_of_chunk0"][chunks[k]]
                                if g > waited:
                                    pe.wait_ge(ga[g], 16)
                                    waited = g
                            else:
                                g = chunks[k] // GRP
                                if g > waited:
                                    a, b_ = st["q1_groups"][g]
                                    pe.wait_ge(gb[g], 16 * (b_ - a))
                                    waited = g
                        mm = pe.matmul(
                            out=psum[T % 4][bh][:],
                            lhsT=st_sb[:, T % 2, k * 128:(k + 1) * 128],
                            rhs=pool_sb[:, ring[k], bh * 512:(bh + 1) * 512],
                            start=(k == 0),
                            stop=(k == kT - 1),
                        )
                        if bh == 1 and k == kT - 1:
                            mm.then_inc(pe_sem, 1)

        @block.vector
        def _(vector: bass.BassEngine):
            for T in range(NT):
                vector.wait_ge(pe_sem, T + 1)
                if T >= 2:
                    vector.wait_ge(yd_sems[T % 2], 16 * (T // 2))
                vector.tensor_copy(out=out_sb[:, T % 2, 0:512],
                                   in_=psum[T % 4][0][:])
                vector.tensor_copy(
                    out=out_sb[:, T % 2, 512:1024], in_=psum[T % 4][1][:]
                ).then_inc(v_sem, 1)

        # stat loads + output stores share the Activation HWDGE
        @block.scalar
        def _(scalar: bass.BassEngine):
            kT = st["k_T"]
            offs = st["offs_T"]
            for T in range(NT):
                if T >= 2:
                    scalar.wait_ge(pe_sem, T - 1)
                scalar.dma_start(
                    out=st_sb[:, T % 2, 0:kT[T] * 128],
                    in_=st_in[:, offs[T] * 128:(offs[T] + kT[T]) * 128],
                ).then_inc(st_sems[T % 2], 16)
                if T >= 2:
                    Ty = T - 2
                    scalar.wait_ge(v_sem, Ty + 1)
                    scalar.dma_start(
                        out=y_out[Ty], in_=out_sb[:, Ty % 2]
                    ).then_inc(yd_sems[Ty % 2], 16)
            for Ty in (NT - 2, NT - 1):
                scalar.wait_ge(v_sem, Ty + 1)
                scalar.dma_start(
                    out=y_out[Ty], in_=out_sb[:, Ty % 2]
                ).then_inc(yd_sems[Ty % 2], 16)
            scalar.wait_ge(yd_sems[0], 16 * (NT // 2))
            scalar.wait_ge(yd_sems[1], 16 * (NT // 2))

    return nc


def _struct_key(st):
    return (
        st["n0"], st["n1"], tuple(st["k_T"]),
        tuple(tuple(r) for r in st["tile_ring"]),
        tuple(st["q1_slot"]), tuple(st["q1_pe_gate"]),
        tuple(st["q1_ga_gate"]), tuple(st["q0_subs"]),
    )


def kernel(x, connections, weights):
    x = np.asarray(x)
    connections = np.asarray(connections)
    weights = np.asarray(weights)

    ckey = (connections.tobytes(), weights.tobytes())
    sp = _static_prep.get(ckey)
    if sp is None:
        sp = _prep_static(connections)
        sp["stat"] = _prep_stat(sp, weights)
        _static_prep.clear()
        _static_prep[ckey] = sp
    st = sp["struct"]

    xT16 = np.ascontiguousarray(x.T.astype(np.float16))        # [IN, B]
    key = _struct_key(st)
    if key not in _cached:
        _cached[key] = _build_program(st)
    nc = _cached[key]

    in_maps = [
        {
            "xTp": np.ascontiguousarray(xT16[sp["perms"][c]]),
            "stat": sp["stat"][c],
            "gidx": sp["gidx"][c],
        }
        for c in range(NCORES)
    ]
    res = run_bass_kernel_spmd(nc, in_maps, core_ids=list(range(NCORES)))
    out = np.empty((B, O), dtype=np.float32)
    for c in range(NCORES):
        y = res.results[c]["y"]                    # [NT, 128, B] f16
        out[:, c * OC:(c + 1) * OC] = (
            y.astype(np.float32).reshape(OC, B).T
        )
    return out


# revision 40
# speedup vs baseline: 1.9294x; 1.1539x over previous
"""PositionalSparseLinear v6: dual-slab quad pools, chunk-major PE,
per-tile trimmed scatter-matrix matmuls.

Per core (1024 outs = 8 tiles of 128), outputs split into two quads of 4
tiles. Each quad's needed input rows (dedup'd) form a pool ordered by the
4-bit "which tiles use this row" class mask in a fixed prefix-staggered
sequence (runs per tile A:1 B:1 C:2 D:4 — the DP optimum), so each tile's
matmuls only cover ~27-31 of the pool's ~59 128-row chunks.

Both pools are CONTIGUOUS SLABS of per-core row-permuted copies of xT
(permutations depend only on `connections`, applied to the activation on
host like the transpose/f16 cast), so all x data loads with plain HWDGE
dma_starts — no Pool-engine descriptor generation at all. Slabs land in a
66-slot SBUF ring; quad 1 reuses quad 0's slots in thr-sorted waves gated
on per-tile PE completion semaphores.

PE runs CHUNK-MAJOR: for each delivered chunk, all tiles using it issue
their matmuls (into per-tile psum banks), matching PE demand to DMA
delivery so the tensor engine stays continuously busy (p-state friendly).
Stat matrices load in two halves per tile, interleaved into the slab
stream in needed-by order.

SPMD constraint: class-region capacities are fixed to the max over cores
(compile-time), so all 8 cores share one program; per-core data (stat
values, permuted x copies) fills the regions.
"""

import sys

sys.path.insert(0, "/opt/trn_rl_repo")

import numpy as np

from contextlib import ExitStack

import concourse.bass as bass
import concourse.mybir as mybir
from concourse.bass_utils import run_bass_kernel_spmd

B = 1024
IN = 8192
O = 8192
K = 32
NCORES = 8
OC = O // NCORES       # 1024
NT = OC // 128         # 8 tiles/core
RING = 66              # pool ring slots (x2KB/partition = 132KB)
GRP = 8                # max chunks per slab sub-op
# stat-load split ordinal per tile-in-quad (None = load whole upfront);
# chosen at each tile's window-run gap so the b-half isn't needed early
SPLITS = [None, 20, 10, 12]

F16 = mybir.dt.float16
F32 = mybir.dt.float32
I32 = mybir.dt.int32

# Class order for 4-bit tile-usage masks (bit t = tile t uses the row).
# Prefix-staggered: tile0's 8 classes first (1 run), tile1 contiguous
# (1 run), tile2 2 runs, tile3 4 runs; staggered finish positions.
CLASS_ORDER = [1, 9, 13, 5, 7, 15, 11, 3, 2, 10, 14, 6, 4, 12, 8]

_cached = {}
_static_prep = {}


def _tile_windows(caps):
    """Per-tile (row_lo, row_hi) run list over class regions with fixed
    capacities `caps` (len 15). Returns [tile][(lo, hi)...] in rows."""
    offs = np.concatenate([[0], np.cumsum(caps)])
    wins = []
    for t in range(4):
        runs = []
        for i, m in enumerate(CLASS_ORDER):
            if (m >> t) & 1:
                lo, hi = offs[i], offs[i + 1]
                if runs and runs[-1][1] == lo:
                    runs[-1] = (runs[-1][0], hi)
                else:
                    runs.append((lo, hi))
        wins.append(runs)
    return wins


def _chunks_of(runs):
    cs = set()
    for lo, hi in runs:
        cs.update(range(lo // 128, (hi + 127) // 128))
    return sorted(cs)


def _prep_static(connections):
    """Everything that depends only on `connections`: permutations, pool
    slot maps, and the shared program structure."""
    conn = connections.reshape(NCORES, NT, 128, K)

    used = np.zeros((NCORES, 2, IN), dtype=np.uint8)
    for c in range(NCORES):
        for T in range(NT):
            q, t = divmod(T, 4)
            rows = np.unique(conn[c, T])
            used[c, q, rows] |= np.uint8(1 << t)

    # fixed class capacities per quad index (max over cores)
    caps = np.zeros((2, 15), dtype=np.int64)
    for q in range(2):
        for i, m in enumerate(CLASS_ORDER):
            caps[q, i] = max(
                int((used[c, q] == m).sum()) for c in range(NCORES)
            )
    L = [int(caps[q].sum()) for q in range(2)]
    n = [(L[q] + 127) // 128 for q in range(2)]
    n0, n1 = n
    assert max(n0, n1) <= RING - 4, (n0, n1)

    wins = [_tile_windows(caps[q]) for q in range(2)]
    tile_chunks = []
    for T in range(NT):
        q, t = divmod(T, 4)
        tile_chunks.append(_chunks_of(wins[q][t]))
    k_T = [len(cs) for cs in tile_chunks]
    TOT = sum(k_T)
    offs_T = np.concatenate([[0], np.cumsum(k_T)])

    # q0 chunk s's last consumer tile (global idx)+1; 0 = no consumer
    thr = np.zeros(n0, dtype=np.int64)
    for t in range(4):
        for j in tile_chunks[t]:
            thr[j] = max(thr[j], t + 1)

    # q0 slab sub-ops: small first ops for an early PE start
    q0_subs = []
    a = 0
    for w in [2, 4] + [GRP] * 100:
        if a >= n0:
            break
        q0_subs.append((a, min(a + w, n0)))
        a += w
    sub_of_chunk0 = np.zeros(n0, dtype=np.int64)
    for k, (a, b) in enumerate(q0_subs):
        sub_of_chunk0[a:b] = k

    # ring slots for q1 chunks: fresh slots first, then q0 slots sorted by
    # (last-consumer thr, chunk) so release waves are monotone in the PE
    # consumption order.
    fresh = list(range(n0, RING))
    q0_slots = sorted(range(n0), key=lambda s: (thr[s], s))
    slot_seq = fresh + q0_slots
    assert n1 <= len(slot_seq)
    q1_slot = [int(s) for s in slot_seq[:n1]]

    # q1 slab sub-ops: maximal runs of consecutive ring slots within the
    # consumption order, capped at GRP chunks. Each sub-op carries its
    # pe gate (max thr of its slots) and ga gate (write-after-write on the
    # q0 slab sub that wrote those slots).
    q1_subs = []          # (j_lo, j_hi, slot_lo, pe_gate, ga_gate)
    j = 0
    while j < n1:
        j0 = j
        s0 = q1_slot[j]
        while (j + 1 < n1 and j + 1 - j0 < GRP
               and q1_slot[j + 1] == q1_slot[j] + 1):
            j += 1
        j += 1
        slots = q1_slot[j0:j]
        pe_gate = max((int(thr[s]) if s < n0 else 0) for s in slots)
        ga_gate = max((int(sub_of_chunk0[s]) if s < n0 else -1)
                      for s in slots)
        q1_subs.append((j0, j, s0, pe_gate, ga_gate))
    sub_of_chunk1 = np.zeros(n1, dtype=np.int64)
    for k, (a, b, *_rest) in enumerate(q1_subs):
        sub_of_chunk1[a:b] = k

    # chunk-major PE schedule: per quad, per chunk -> [(T, k, first, last)]
    chunk_sched = []
    for q in range(2):
        per_chunk = [[] for _ in range(n[q])]
        for t in range(4):
            T = 4 * q + t
            cs = tile_chunks[T]
            for k, jj in enumerate(cs):
                per_chunk[jj].append((T, k, k == 0, k == len(cs) - 1))
        chunk_sched.append(per_chunk)
    qoff = []
    for T in range(NT):
        base = 0 if T // 4 == 0 else 4
        qoff.append(sum(k_T[base:T]) if T > base else 0)
    QSTAT = max(sum(k_T[0:4]), sum(k_T[4:8]))

    # per-core pools and permutations
    pools = np.zeros((NCORES, 2, max(n0, n1) * 128), dtype=np.int64)
    perms = np.zeros((NCORES, 2, IN), dtype=np.int64)
    slot = np.full((NCORES, 2, IN), -1, dtype=np.int64)
    for c in range(NCORES):
        for q in range(2):
            Lq = n[q] * 128
            pool = np.zeros(Lq, dtype=np.int64)
            fill = np.zeros(Lq, dtype=bool)
            off = 0
            for i, m in enumerate(CLASS_ORDER):
                rows = np.flatnonzero(used[c, q] == m)
                pool[off:off + len(rows)] = rows
                fill[off:off + len(rows)] = True
                off += int(caps[q, i])
            unused = np.flatnonzero(used[c, q] == 0)
            npad = int((~fill).sum())
            assert npad <= len(unused), (c, q, npad, len(unused))
            pool[~fill] = unused[:npad]
            pools[c, q, :Lq] = pool
            real = np.flatnonzero(fill)
            slot[c, q, pool[real]] = real
            inp = np.zeros(IN, dtype=bool)
            inp[pool] = True
            perms[c, q] = np.concatenate([pool, np.flatnonzero(~inp)])

    struct = dict(
        n0=n0, n1=n1, k_T=k_T, TOT=TOT,
        offs_T=[int(v) for v in offs_T],
        tile_chunks=tile_chunks,
        q1_slot=q1_slot,
        q0_subs=q0_subs, q1_subs=q1_subs,
        sub_of_chunk0=[int(v) for v in sub_of_chunk0],
        sub_of_chunk1=[int(v) for v in sub_of_chunk1],
        chunk_sched=chunk_sched, qoff=qoff, QSTAT=QSTAT,
    )
    return dict(conn=conn, used=used, caps=caps, struct=struct,
                pools=pools, perms=perms, slot=slot, n=n)


def _prep_stat(sp, weights):
    """Per-core packed stat matrices [128, TOT*128] f16."""
    st = sp["struct"]
    conn, slot = sp["conn"], sp["slot"]
    wts = weights.reshape(NCORES, NT, 128, K).astype(np.float32)
    n = sp["n"]
    stat = np.zeros((NCORES, 128, st["TOT"] * 128), dtype=np.float16)
    m_idx = np.repeat(np.arange(128), K).reshape(128, K)
    for c in range(NCORES):
        for T in range(NT):
            q = T // 4
            s = slot[c, q][conn[c, T]]
            assert (s >= 0).all()
            acc = np.zeros((n[q] * 128, 128), dtype=np.float32)
            np.add.at(acc, (s, m_idx), wts[c, T])
            blk = acc.reshape(n[q], 128, 128)[st["tile_chunks"][T]]
            blk = blk.transpose(1, 0, 2).reshape(128, -1)
            o = st["offs_T"][T] * 128
            stat[c, :, o:o + blk.shape[1]] = blk.astype(np.float16)
    return stat


def _build_program(st):
    n0, n1 = st["n0"], st["n1"]
    nc = bass.Bass()
    xp0_in = nc.declare_dram_parameter("xTp0", [IN, B], F16, isOutput=False)
    xp1_in = nc.declare_dram_parameter("xTp1", [IN, B], F16, isOutput=False)
    st_in = nc.declare_dram_parameter("stat", [128, st["TOT"] * 128], F16,
                                      isOutput=False)
    y_out = nc.declare_dram_parameter("y", [NT, 128, B], F16, isOutput=True)

    nga = len(st["q0_subs"])
    ngb = len(st["q1_subs"])

    with (
        nc.sbuf_tensor("pool_sb", [128, RING, B], F16) as pool_sb,
        nc.sbuf_tensor("st_sb", [128, 2, st["QSTAT"] * 128], F16) as st_sb,
        nc.sbuf_tensor("out_sb", [128, 4, B], F16) as out_sb,
        ExitStack() as _stack,
        nc.Block() as block,
        nc.semaphore("pe_sem") as pe_sem,
        nc.semaphore("v_sem") as v_sem,
        nc.semaphore("yd_sem0") as yd_sem0,
        nc.semaphore("yd_sem1") as yd_sem1,
    ):
        ga = [_stack.enter_context(nc.semaphore(f"ga{i}")) for i in range(nga)]
        gb = [_stack.enter_context(nc.semaphore(f"gb{i}")) for i in range(ngb)]
        stq = [_stack.enter_context(nc.semaphore(f"stq{i}"))
               for i in range(NT)]
        _ps = [
            _stack.enter_context(nc.psum_tensor(f"ps{i}", [128, 512], F32))
            for i in range(8)
        ]
        psum = [(_ps[0], _ps[1]), (_ps[2], _ps[3]), (_ps[4], _ps[5]),
                (_ps[6], _ps[7])]
        yd_sems = [yd_sem0, yd_sem1]

        def _stat_load(eng, T, half):
            kT = st["k_T"][T]
            o = st["qoff"][T] * 128
            offs = st["offs_T"][T] * 128
            sp_ = SPLITS[T % 4]
            if sp_ is None:
                if half == 1:
                    return
                lo, hi = 0, kT
            else:
                lo, hi = (0, min(sp_, kT)) if half == 0 else (sp_, kT)
            if lo >= hi:
                return
            eng.dma_start(
                out=st_sb[:, T // 4, o + lo * 128:o + hi * 128],
                in_=st_in[:, offs + lo * 128:offs + hi * 128],
            ).then_inc(stq[T], 16)

        def _slab(eng, xp, a, b, s0, sem):
            src = xp[a * 128:b * 128].rearrange("(j s) b -> s j b", s=128)
            eng.dma_start(
                out=pool_sb[:, s0:s0 + (b - a), :], in_=src
            ).then_inc(sem, 16)

        # one engine = one ordered arrival stream at the DMA device,
        # sequenced by PE needed-by times.
        @block.sync
        def _(sync: bass.BassEngine):
            stat_a_after = {-1: [0], 2: [3, 2, 1]}
            stat_b_after = {4: [3, 2, 1]}
            for T in stat_a_after[-1]:
                _stat_load(sync, T, 0)
            for k, (a, b) in enumerate(st["q0_subs"]):
                _slab(sync, xp0_in, a, b, a, ga[k])
                for T in stat_a_after.get(k, []):
                    _stat_load(sync, T, 0)
                for T in stat_b_after.get(k, []):
                    _stat_load(sync, T, 1)
            # q1 slab sub-ops, gated on ring-slot availability
            last_pe = 0
            waited_ga = set()
            for k, (a, b, s0, pe_gate, ga_gate) in enumerate(st["q1_subs"]):
                if ga_gate >= 0 and ga_gate not in waited_ga:
                    sync.wait_ge(ga[ga_gate], 16)
                    waited_ga.add(ga_gate)
                if pe_gate > last_pe:
                    sync.wait_ge(pe_sem, pe_gate)
                    last_pe = pe_gate
                _slab(sync, xp1_in, a, b, s0, gb[k])

        # chunk-major PE: for each delivered chunk, run all tiles using it
        @block.tensor
        def _(pe: bass.BassEngine):
            for q in range(2):
                nq = st["n0"] if q == 0 else st["n1"]
                sched = st["chunk_sched"][q]
                sub_of = st["sub_of_chunk0"] if q == 0 else \
                    st["sub_of_chunk1"]
                gsem = ga if q == 0 else gb
                waited = -1
                if q == 0:
                    # warmup: bank the first 3 slab subs (14 chunks) before
                    # the first matmul so the early phase never starves
                    for gg in range(min(2, nga - 1) + 1):
                        pe.wait_ge(ga[gg], 16)
                    waited = min(2, nga - 1)
                nsub = len(st["q0_subs"] if q == 0 else st["q1_subs"])
                for j in range(nq):
                    if not sched[j]:
                        continue
                    g = sub_of[j]
                    if g > waited:
                        pe.wait_ge(gsem[g], 16)
                        waited = g
                    for (T, k, first, last) in sched[j]:
                        if first:
                            pe.wait_ge(stq[T], 16)
                            if T >= 4:
                                pe.wait_ge(v_sem, T - 3)
                        sp_ = SPLITS[T % 4]
                        if sp_ is not None and k == sp_:
                            pe.wait_ge(stq[T], 32)
                        slot = (j if q == 0 else st["q1_slot"][j])
                        col = (st["qoff"][T] + k) * 128
                        for bh in range(2):
                            mm = pe.matmul(
                                out=psum[T % 4][bh][:],
                                lhsT=st_sb[:, q, col:col + 128],
                                rhs=pool_sb[:, slot,
                                            bh * 512:(bh + 1) * 512],
                                start=first,
                                stop=last,
                            )
                            if last and bh == 1:
                                mm.then_inc(pe_sem, 1)

        @block.vector
        def _(vector: bass.BassEngine):
            for T in range(NT):
                vector.wait_ge(pe_sem, T + 1)
                if T >= 4:
                    vector.wait_ge(yd_sems[T % 2], 16 * ((T - 4) // 2 + 1))
                vector.tensor_copy(out=out_sb[:, T % 4, 0:512],
                                   in_=psum[T % 4][0][:])
                vector.tensor_copy(
                    out=out_sb[:, T % 4, 512:1024], in_=psum[T % 4][1][:]
                ).then_inc(v_sem, 1)

        # q1 stats (held until the q0 slab is in) + output stores
        @block.scalar
        def _(scalar: bass.BassEngine):
            scalar.wait_ge(ga[nga - 1], 16)
            for T in range(4, NT):
                _stat_load(scalar, T, 0)
                _stat_load(scalar, T, 1)
            for T in range(NT):
                scalar.wait_ge(v_sem, T + 1)
                scalar.dma_start(
                    out=y_out[T], in_=out_sb[:, T % 4]
                ).then_inc(yd_sems[T % 2], 16)
            scalar.wait_ge(yd_sems[0], 16 * (NT // 2))
            scalar.wait_ge(yd_sems[1], 16 * (NT // 2))

    return nc


def _struct_key(st):
    return (
        st["n0"], st["n1"], tuple(st["k_T"]),
        tuple(st["q1_slot"]), tuple(st["q0_subs"]),
        tuple(st["q1_subs"]),
    )


def kernel(x, connections, weights):
    x = np.asarray(x)
    connections = np.asarray(connections)
    weights = np.asarray(weights)

    ckey = (connections.tobytes(), weights.tobytes())
    sp = _static_prep.get(ckey)
    if sp is None:
        sp = _prep_static(connections)
        sp["stat"] = _prep_stat(sp, weights)
        _static_prep.clear()
        _static_prep[ckey] = sp
    st = sp["struct"]

    xT16 = np.ascontiguousarray(x.T.astype(np.float16))        # [IN, B]
    key = _struct_key(st)
    if key not in _cached:
        _cached[key] = _build_program(st)
    nc = _cached[key]

    in_maps = [
        {
            "xTp0": np.ascontiguousarray(xT16[sp["perms"][c, 0]]),
            "xTp1": np.ascontiguousarray(xT16[sp["perms"][c, 1]]),
            "stat": sp["stat"][c],
        }
        for c in range(NCORES)
    ]
    res = run_bass_kernel_spmd(nc, in_maps, core_ids=list(range(NCORES)))
    out = np.empty((B, O), dtype=np.float32)
    for c in range(NCORES):
        y = res.results[c]["y"]                    # [NT, 128, B] f16
        out[:, c * OC:(c + 1) * OC] = (
            y.astype(np.float32).reshape(OC, B).T
        )
    return out
